# revision 3
# baseline (speedup 1.0000x reference)
"""Trainium2 Bass kernel for NodeTimeSeriesDecoder (per-node 2-layer LSTM over T=256).

Strategy: data-parallel over graphs across 8 cores (whole graphs -> contiguous
node blocks). Per core: nodes padded per-graph to multiples of B=512 columns.
All per-(graph, t) terms (encoder latent/gms mix, layer biases) are precomputed
on device into small tables; the per-timestep work is only the node-dependent
matmuls + LSTM pointwise ops. Output is produced node-major via transposed
decoder matmuls so DRAM writes are contiguous per node.
"""
import sys
sys.path.insert(0, "/opt/trn_rl_repo")
import numpy as np
import concourse.bass as bass
import concourse.bacc as bacc
import concourse.tile as tile
from concourse import mybir
from concourse.bass_utils import run_bass_kernel_spmd

F32 = mybir.dt.float32
F32R = mybir.dt.float32r
AF = mybir.ActivationFunctionType
DS = bass.DynSlice

H = 128          # lstm hidden / graph latent dim
T = 256          # timesteps
GM = 3           # ground-motion dim
ND = 6           # node feature dim
B = 512          # node columns per block
NCORES = 8
UNROLL = 8

GATE_FUNCS = [AF.Sigmoid, AF.Sigmoid, AF.Tanh, AF.Sigmoid]  # i, f, g, o


def build_nc(NBLK, NTAB, blkmap):
    NPAD = NBLK * B
    nc = bacc.Bacc(None, target_bir_lowering=False)

    node_t_ext = nc.declare_dram_parameter("node_t", [ND, NPAD], F32R, isOutput=False)
    latT_ext = nc.declare_dram_parameter("latT", [NTAB, H, T], F32R, isOutput=False)
    gmsT_ext = nc.declare_dram_parameter("gmsT", [NTAB, GM, T], F32R, isOutput=False)
    wencT_node_ext = nc.declare_dram_parameter("wencT_node", [ND, H], F32R, isOutput=False)
    wencT_lat_ext = nc.declare_dram_parameter("wencT_lat", [H, H], F32R, isOutput=False)
    wencT_gm_ext = nc.declare_dram_parameter("wencT_gm", [GM, H], F32R, isOutput=False)
    wihT0_ext = nc.declare_dram_parameter("wihT0", [H, 4 * H], F32R, isOutput=False)
    whhT0_ext = nc.declare_dram_parameter("whhT0", [H, 4 * H], F32R, isOutput=False)
    wihT1_ext = nc.declare_dram_parameter("wihT1", [H, 4 * H], F32R, isOutput=False)
    wihT1gm_ext = nc.declare_dram_parameter("wihT1gm", [GM, 4 * H], F32R, isOutput=False)
    whhT1_ext = nc.declare_dram_parameter("whhT1", [H, 4 * H], F32R, isOutput=False)
    wd1T_ext = nc.declare_dram_parameter("wd1T", [H, 2, 64], F32R, isOutput=False)
    wd2T_ext = nc.declare_dram_parameter("wd2T", [64, 4], F32R, isOutput=False)
    gbias_ext = nc.declare_dram_parameter("gbias", [H, 8], F32, isOutput=False)  # (bih+bhh) l0c0..3, l1c0..3
    benc_ext = nc.declare_dram_parameter("benc", [H, 1], F32, isOutput=False)
    bd1_ext = nc.declare_dram_parameter("bd1", [64, 1], F32, isOutput=False)
    bd2rep_ext = nc.declare_dram_parameter("bd2rep", [H, GM], F32, isOutput=False)
    y_ext = nc.declare_dram_parameter("y", [NPAD, T, GM], F32, isOutput=True)

    with tile.TileContext(nc) as tc:
        with tc.tile_pool(name="consts", bufs=1) as cp, \
             tc.tile_pool(name="work", bufs=1) as wp, \
             tc.tile_pool(name="ps", bufs=8, space="PSUM") as pp:

            # ---- load constants ----
            wencT_node = cp.tile([ND, H], F32R)
            wencT_lat = cp.tile([H, H], F32R)
            wencT_gm = cp.tile([GM, H], F32R)
            wihT0 = cp.tile([H, 4 * H], F32R)
            whhT0 = cp.tile([H, 4 * H], F32R)
            wihT1 = cp.tile([H, 4 * H], F32R)
            wihT1gm = cp.tile([GM, 4 * H], F32R)
            whhT1 = cp.tile([H, 4 * H], F32R)
            wd1T = cp.tile([H, 2, 64], F32R)
            wd2T = cp.tile([64, 4], F32R)
            gbias = cp.tile([H, 8], F32)
            benc = cp.tile([H, 1], F32)
            bd1 = cp.tile([64, 1], F32)
            bd2rep = cp.tile([H, GM], F32)
            latT_all = cp.tile([H, NTAB, T], F32R)
            gmsT_all = cp.tile([GM, NTAB, T], F32R)
            for dst, src in [(wencT_node, wencT_node_ext), (wencT_lat, wencT_lat_ext),
                             (wencT_gm, wencT_gm_ext), (wihT0, wihT0_ext),
                             (whhT0, whhT0_ext), (wihT1, wihT1_ext), (wihT1gm, wihT1gm_ext), (whhT1, whhT1_ext),
                             (wd1T, wd1T_ext), (wd2T, wd2T_ext), (gbias, gbias_ext),
                             (benc, benc_ext), (bd1, bd1_ext), (bd2rep, bd2rep_ext)]:
                nc.sync.dma_start(out=dst, in_=src[:])
            for g in range(NTAB):
                nc.sync.dma_start(out=latT_all[:, g, :], in_=latT_ext[g])
                nc.sync.dma_start(out=gmsT_all[:, g, :], in_=gmsT_ext[g])

            # ---- per-(tab, t) tables ----
            # E[tab][:, t]   = Wlat @ lat[g,t] + Wgm @ gms[g,t] + b_enc          [H, T]
            # M12[tab][:, c, t] c=0..3: Wih0_c @ E[:, t] + bias0_c              (layer-1 gate bias)
            # M12[tab][:, 4+c, t]:      Wih1_c[:, 125:] @ gms[g,t] + bias1_c    (layer-2 gate bias)
            E_f = cp.tile([H, NTAB, T], F32)
            E_r = cp.tile([H, NTAB, T], F32R)
            M12 = cp.tile([H, NTAB, 8, T], F32)
            for g in range(NTAB):
                pe = pp.tile([H, T], F32, tag="ps", name=f"pe{g}")
                nc.tensor.matmul(pe, wencT_lat, latT_all[:, g, :], start=True, stop=False)
                nc.tensor.matmul(pe, wencT_gm, gmsT_all[:, g, :], start=False, stop=True)
                nc.vector.tensor_scalar_add(E_f[:, g, :], pe, benc)
                nc.vector.tensor_scalar_add(E_r[:, g, :], pe, benc)
                for c in range(4):
                    pm = pp.tile([H, T], F32, tag="ps", name=f"pm{g}_{c}")
                    nc.tensor.matmul(pm, wihT0[:, c * H:(c + 1) * H], E_r[:, g, :],
                                     start=True, stop=True)
                    nc.vector.tensor_scalar_add(M12[:, g, c, :], pm, gbias[:, c:c + 1])
                for c in range(4):
                    pm2 = pp.tile([H, T], F32, tag="ps", name=f"pm2{g}_{c}")
                    nc.tensor.matmul(pm2, wihT1gm[:, c * H:(c + 1) * H],
                                     gmsT_all[:, g, :], start=True, stop=True)
                    nc.vector.tensor_scalar_add(M12[:, g, 4 + c, :], pm2,
                                                gbias[:, 4 + c:5 + c])

            # ---- main block loop ----
            for b in range(NBLK):
                g = blkmap[b]
                nb = wp.tile([ND, B], F32R, tag="nb", bufs=2, name=f"nb{b}")
                nc.sync.dma_start(out=nb, in_=node_t_ext[:, b * B:(b + 1) * B])

                px = pp.tile([H, B], F32, tag="ps", name=f"px{b}")
                nc.tensor.matmul(px, wencT_node, nb, start=True, stop=True)
                enc_node = wp.tile([H, B], F32R, tag="encn", bufs=2, name=f"encn{b}")
                nc.vector.tensor_copy(enc_node, px)

                # persistent ping-pong state tiles for this block
                h1 = [wp.tile([H, B], F32R, tag=f"h1_{i}", bufs=2, name=f"h1_{i}_{b}") for i in range(2)]
                c1 = [wp.tile([H, B], F32, tag=f"c1_{i}", bufs=2, name=f"c1_{i}_{b}") for i in range(2)]
                h2 = [wp.tile([H, B], F32R, tag=f"h2_{i}", bufs=2, name=f"h2_{i}_{b}") for i in range(2)]
                c2 = [wp.tile([H, B], F32R, tag=f"c2_{i}", bufs=2, name=f"c2_{i}_{b}") for i in range(2)]
                for dst in (h1[0], c1[0], h2[0], c2[0]):
                    nc.vector.tensor_scalar_add(dst, px, E_f[:, g, 0:1])

                yacc = [wp.tile([H, T, GM], F32, tag=f"yacc{j}", bufs=2, name=f"yacc{j}_{b}")
                        for j in range(4)]
                stg = [wp.tile([H, 8], F32, tag=f"stg{i}", bufs=2, name=f"stg{i}_{b}")
                       for i in range(2)]

                with tc.For_i(0, T, UNROLL) as tv:
                    for k in range(UNROLL):
                        t = tv + k
                        pcur, pnxt = k % 2, 1 - (k % 2)
                        nc.vector.tensor_copy(stg[pcur], M12[:, g, :, DS(t, 1)].squeeze(-1))

                        # ---- layer 1 gates ----
                        acts1 = []
                        for c in range(4):
                            pg = pp.tile([H, B], F32, tag="ps", name=f"g1_{c}")
                            nc.tensor.matmul(pg, whhT0[:, c * H:(c + 1) * H], h1[pcur],
                                             start=True, stop=False)
                            nc.tensor.matmul(pg, wihT0[:, c * H:(c + 1) * H], enc_node,
                                             start=False, stop=True)
                            a = wp.tile([H, B], F32, tag=f"a1_{c}", bufs=2, name=f"a1_{c}")
                            nc.scalar.activation(out=a, in_=pg, func=GATE_FUNCS[c],
                                                 bias=stg[pcur][:, c:c + 1], scale=1.0)
                            acts1.append(a)
                        t1 = wp.tile([H, B], F32, tag="t1", bufs=2, name="t1")
                        t2 = wp.tile([H, B], F32, tag="t2", bufs=2, name="t2")
                        nc.vector.tensor_mul(t1, acts1[0], acts1[2])      # sig(i)*tanh(g)
                        nc.vector.tensor_mul(t2, acts1[1], c1[pcur])      # sig(f)*c
                        nc.vector.tensor_add(c1[pnxt], t1, t2)
                        tc1 = wp.tile([H, B], F32, tag="tc1", bufs=2, name="tc1")
                        nc.scalar.activation(out=tc1, in_=c1[pnxt], func=AF.Tanh)
                        nc.vector.tensor_mul(h1[pnxt], acts1[3], tc1)     # sig(o)*tanh(c)

                        # ---- layer 2 gates (input = [h1_new[:125]; gm] via M2 bias) ----
                        acts2 = []
                        for c in range(4):
                            pg2 = pp.tile([H, B], F32, tag="ps", name=f"g2_{c}")
                            nc.tensor.matmul(pg2, wihT1[0:H - GM, c * H:(c + 1) * H],
                                             h1[pnxt][0:H - GM, :], start=True, stop=False)
                            nc.tensor.matmul(pg2, whhT1[:, c * H:(c + 1) * H], h2[pcur],
                                             start=False, stop=True)
                            a2 = wp.tile([H, B], F32, tag=f"a2_{c}", bufs=2, name=f"a2_{c}")
                            nc.scalar.activation(out=a2, in_=pg2, func=GATE_FUNCS[c],
                                                 bias=stg[pcur][:, 4 + c:5 + c], scale=1.0)
                            acts2.append(a2)
                        t3 = wp.tile([H, B], F32, tag="t3", bufs=2, name="t3")
                        t4 = wp.tile([H, B], F32, tag="t4", bufs=2, name="t4")
                        nc.vector.tensor_mul(t3, acts2[0], acts2[2])
                        nc.vector.tensor_mul(t4, acts2[1], c2[pcur])
                        nc.vector.tensor_add(c2[pnxt], t3, t4)
                        tc2 = wp.tile([H, B], F32, tag="tc2", bufs=2, name="tc2")
                        nc.scalar.activation(out=tc2, in_=c2[pnxt], func=AF.Tanh)
                        nc.vector.tensor_mul(h2[pnxt], acts2[3], tc2)

                        # ---- decoder ----
                        pd = pp.tile([64, B], F32, tag="ps", name="pd")
                        nc.tensor.matmul(pd, wd1T[:, 0, :], h2[pnxt], start=True, stop=False)
                        nc.tensor.matmul(pd, wd1T[:, 1, :], c2[pnxt], start=False, stop=True)
                        relu = wp.tile([64, B], F32R, tag="relu", bufs=2, name="relu")
                        nc.scalar.activation(out=relu, in_=pd, func=AF.Relu, bias=bd1, scale=1.0)
                        py = pp.tile([H, 16], F32, tag="ps", name="py")
                        for j in range(4):
                            nc.tensor.matmul(py[:, j * 4:(j + 1) * 4],
                                             relu[:, j * H:(j + 1) * H], wd2T,
                                             start=True, stop=True)
                        for j in range(4):
                            nc.vector.tensor_add(
                                yacc[j][:, DS(t, 1), :].squeeze(1),
                                py[:, j * 4:j * 4 + GM], bd2rep)

                for j in range(4):
                    nc.sync.dma_start(out=y_ext[b * B + j * H:b * B + (j + 1) * H, :, :],
                                      in_=yacc[j])

    nc.finalize()
    return nc


_CACHE = {}


def _get_nc(NBLK, NTAB, blkmap):
    key = (NBLK, NTAB, tuple(blkmap))
    if key not in _CACHE:
        _CACHE[key] = build_nc(NBLK, NTAB, blkmap)
    return _CACHE[key]


def kernel(node, ptr, graph_time_series_behavior, ground_motions,
           W_enc, b_enc, W_ih, W_hh, b_ih, b_hh, W_d1, b_d1, W_d2, b_d2):
    node = np.asarray(node, np.float32)
    ptr = np.asarray(ptr, np.int64)
    lat = np.asarray(graph_time_series_behavior, np.float32)
    gms = np.asarray(ground_motions, np.float32)
    W_enc = np.asarray(W_enc, np.float32); b_enc_a = np.asarray(b_enc, np.float32)
    W_ih = np.asarray(W_ih, np.float32); W_hh = np.asarray(W_hh, np.float32)
    b_ih = np.asarray(b_ih, np.float32); b_hh = np.asarray(b_hh, np.float32)
    W_d1 = np.asarray(W_d1, np.float32); b_d1_a = np.asarray(b_d1, np.float32)
    W_d2 = np.asarray(W_d2, np.float32); b_d2_a = np.asarray(b_d2, np.float32)

    N = node.shape[0]
    BS = lat.shape[0]
    gsizes = np.diff(ptr).astype(np.int64)
    assert gsizes.sum() == N

    # assign whole graphs to cores (contiguous groups)
    gper = (BS + NCORES - 1) // NCORES
    core_graphs = [list(range(c * gper, min((c + 1) * gper, BS))) for c in range(NCORES)]
    NTAB = max(len(cg) for cg in core_graphs)
    # per-core block -> tab map; blocks are per-graph padded to multiples of B
    core_blkmaps, core_nblk = [], []
    for cg in core_graphs:
        bm = []
        for slot, g in enumerate(cg):
            bm += [slot] * int((gsizes[g] + B - 1) // B)
        core_blkmaps.append(bm)
        core_nblk.append(len(bm))
    NBLK = max(core_nblk) if max(core_nblk) > 0 else 1
    for bm in core_blkmaps:
        bm += [0] * (NBLK - len(bm))
    if all(bm == core_blkmaps[0] for bm in core_blkmaps):
        blkmap = core_blkmaps[0]
        per_block_tabs = False
    else:
        # ragged fallback: one tab per block, latent replicated per block
        blkmap = list(range(NBLK))
        NTAB = NBLK
        per_block_tabs = True

    NPAD = NBLK * B
    # host-side permutation: per core, concatenated per-graph node ranges padded to B
    latT_w = np.ascontiguousarray(lat.transpose(0, 2, 1))   # [BS, H, T]
    gmsT_w = np.ascontiguousarray(gms.transpose(0, 2, 1))   # [BS, GM, T]

    in_maps = []
    core_index_maps = []
    for c, cg in enumerate(core_graphs):
        node_pad = np.zeros((NPAD, ND), np.float32)
        idx_map = np.full(NPAD, -1, np.int64)
        pos = 0
        for g in cg:
            s, e = int(ptr[g]), int(ptr[g + 1])
            n = e - s
            node_pad[pos:pos + n] = node[s:e]
            idx_map[pos:pos + n] = np.arange(s, e)
            pos += int((n + B - 1) // B) * B
        latT_c = np.zeros((NTAB, H, T), np.float32)
        gmsT_c = np.zeros((NTAB, GM, T), np.float32)
        if per_block_tabs:
            for bi, g_slot in enumerate(core_blkmaps[c][:core_nblk[c]]):
                pass
            # map each block to its graph directly
            bi = 0
            for g in cg:
                nblk_g = int((gsizes[g] + B - 1) // B)
                for _ in range(nblk_g):
                    latT_c[bi] = latT_w[g]
                    gmsT_c[bi] = gmsT_w[g]
                    bi += 1
        else:
            for slot, g in enumerate(cg):
                latT_c[slot] = latT_w[g]
                gmsT_c[slot] = gmsT_w[g]
        in_maps.append(dict(
            node_t=np.ascontiguousarray(node_pad.T),
            latT=latT_c, gmsT=gmsT_c,
            wencT_node=np.ascontiguousarray(W_enc[:, :ND].T),
            wencT_lat=np.ascontiguousarray(W_enc[:, ND:ND + H].T),
            wencT_gm=np.ascontiguousarray(W_enc[:, ND + H:].T),
            wihT0=np.ascontiguousarray(W_ih[0].T),
            whhT0=np.ascontiguousarray(W_hh[0].T),
            wihT1=np.ascontiguousarray(W_ih[1].T),
            wihT1gm=np.ascontiguousarray(W_ih[1][:, H - GM:].T),
            whhT1=np.ascontiguousarray(W_hh[1].T),
            wd1T=np.ascontiguousarray(np.stack([W_d1[:, :H].T, W_d1[:, H:].T], axis=1)),
            wd2T=np.ascontiguousarray(np.concatenate([W_d2.T, np.zeros((64, 1), np.float32)], 1)),
            gbias=np.ascontiguousarray(
                np.concatenate([(b_ih[0] + b_hh[0]).reshape(4, H),
                                (b_ih[1] + b_hh[1]).reshape(4, H)], 0).T),
            benc=b_enc_a.reshape(H, 1),
            bd1=b_d1_a.reshape(64, 1),
            bd2rep=np.ascontiguousarray(np.broadcast_to(b_d2_a, (H, GM))),
        ))
        core_index_maps.append(idx_map)

    nc = _get_nc(NBLK, NTAB, blkmap)
    res = run_bass_kernel_spmd(nc, in_maps, list(range(NCORES)))

    out = np.empty((N, T, GM), np.float32)
    for c in range(NCORES):
        y = res.results[c]["y"]
        m = core_index_maps[c]
        valid = m >= 0
        out[m[valid]] = y[valid]
    return out


# revision 4
# speedup vs baseline: 163.8595x; 163.8595x over previous
"""Trainium2 Bass kernel for NodeTimeSeriesDecoder (per-node 2-layer LSTM over T=256).

Strategy: data-parallel over graphs across 8 cores (whole graphs -> contiguous
node blocks). Per core: nodes padded per-graph to multiples of B=512 columns.
All per-(graph, t) terms (encoder latent/gms mix, layer biases) are precomputed
on device into small tables; the per-timestep work is only the node-dependent
matmuls + LSTM pointwise ops. Output is produced node-major via transposed
decoder matmuls so DRAM writes are contiguous per node.
"""
import sys
sys.path.insert(0, "/opt/trn_rl_repo")
import numpy as np
import concourse.bass as bass
import concourse.bacc as bacc
import concourse.tile as tile
from concourse import mybir
from concourse.bass_utils import run_bass_kernel_spmd

F32 = mybir.dt.float32
F32R = mybir.dt.float32r
AF = mybir.ActivationFunctionType
DS = bass.DynSlice

H = 128          # lstm hidden / graph latent dim
T = 256          # timesteps
GM = 3           # ground-motion dim
ND = 6           # node feature dim
B = 512          # node columns per block
NCORES = 8
UNROLL = 8

GATE_FUNCS = [AF.Sigmoid, AF.Sigmoid, AF.Tanh, AF.Sigmoid]  # i, f, g, o


def build_nc(NBLK, NTAB, blkmap):
    NPAD = NBLK * B
    nc = bacc.Bacc(None, target_bir_lowering=False)

    node_t_ext = nc.declare_dram_parameter("node_t", [ND, NPAD], F32R, isOutput=False)
    latT_ext = nc.declare_dram_parameter("latT", [NTAB, H, T], F32R, isOutput=False)
    gmsT_ext = nc.declare_dram_parameter("gmsT", [NTAB, GM, T], F32R, isOutput=False)
    wencT_node_ext = nc.declare_dram_parameter("wencT_node", [ND, H], F32R, isOutput=False)
    wencT_lat_ext = nc.declare_dram_parameter("wencT_lat", [H, H], F32R, isOutput=False)
    wencT_gm_ext = nc.declare_dram_parameter("wencT_gm", [GM, H], F32R, isOutput=False)
    wihT0_ext = nc.declare_dram_parameter("wihT0", [H, 4 * H], F32R, isOutput=False)
    whhT0_ext = nc.declare_dram_parameter("whhT0", [H, 4 * H], F32R, isOutput=False)
    wihT1_ext = nc.declare_dram_parameter("wihT1", [H, 4 * H], F32R, isOutput=False)
    wihT1gm_ext = nc.declare_dram_parameter("wihT1gm", [GM, 4 * H], F32R, isOutput=False)
    whhT1_ext = nc.declare_dram_parameter("whhT1", [H, 4 * H], F32R, isOutput=False)
    wd1T_ext = nc.declare_dram_parameter("wd1T", [H, 2, 64], F32R, isOutput=False)
    wd2T_ext = nc.declare_dram_parameter("wd2T", [64, 4], F32R, isOutput=False)
    gbias_ext = nc.declare_dram_parameter("gbias", [H, 8], F32, isOutput=False)  # (bih+bhh) l0c0..3, l1c0..3
    benc_ext = nc.declare_dram_parameter("benc", [H, 1], F32, isOutput=False)
    bd1_ext = nc.declare_dram_parameter("bd1", [64, 1], F32, isOutput=False)
    bd2rep_ext = nc.declare_dram_parameter("bd2rep", [H, GM], F32, isOutput=False)
    y_ext = nc.declare_dram_parameter("y", [NPAD, T, GM], F32, isOutput=True)

    with tile.TileContext(nc) as tc:
        with tc.tile_pool(name="consts", bufs=1) as cp, \
             tc.tile_pool(name="work", bufs=1) as wp, \
             tc.tile_pool(name="ps", bufs=8, space="PSUM") as pp:

            # ---- load constants ----
            wencT_node = cp.tile([ND, H], F32R)
            wencT_lat = cp.tile([H, H], F32R)
            wencT_gm = cp.tile([GM, H], F32R)
            wihT0 = cp.tile([H, 4 * H], F32R)
            whhT0 = cp.tile([H, 4 * H], F32R)
            wihT1 = cp.tile([H, 4 * H], F32R)
            wihT1gm = cp.tile([GM, 4 * H], F32R)
            whhT1 = cp.tile([H, 4 * H], F32R)
            wd1T = cp.tile([H, 2, 64], F32R)
            wd2T = cp.tile([64, 4], F32R)
            gbias = cp.tile([H, 8], F32)
            benc = cp.tile([H, 1], F32)
            bd1 = cp.tile([64, 1], F32)
            bd2rep = cp.tile([H, GM], F32)
            latT_all = cp.tile([H, NTAB, T], F32R)
            gmsT_all = cp.tile([GM, NTAB, T], F32R)
            for dst, src in [(wencT_node, wencT_node_ext), (wencT_lat, wencT_lat_ext),
                             (wencT_gm, wencT_gm_ext), (wihT0, wihT0_ext),
                             (whhT0, whhT0_ext), (wihT1, wihT1_ext), (wihT1gm, wihT1gm_ext), (whhT1, whhT1_ext),
                             (wd1T, wd1T_ext), (wd2T, wd2T_ext), (gbias, gbias_ext),
                             (benc, benc_ext), (bd1, bd1_ext), (bd2rep, bd2rep_ext)]:
                nc.sync.dma_start(out=dst, in_=src[:])
            for g in range(NTAB):
                nc.sync.dma_start(out=latT_all[:, g, :], in_=latT_ext[g])
                nc.sync.dma_start(out=gmsT_all[:, g, :], in_=gmsT_ext[g])

            # ---- per-(tab, t) tables ----
            # E[tab][:, t]   = Wlat @ lat[g,t] + Wgm @ gms[g,t] + b_enc          [H, T]
            # M12[tab][:, c, t] c=0..3: Wih0_c @ E[:, t] + bias0_c              (layer-1 gate bias)
            # M12[tab][:, 4+c, t]:      Wih1_c[:, 125:] @ gms[g,t] + bias1_c    (layer-2 gate bias)
            E_f = cp.tile([H, NTAB, T], F32)
            E_r = cp.tile([H, NTAB, T], F32R)
            M12 = cp.tile([H, NTAB, 8, T], F32)
            for g in range(NTAB):
                pe = pp.tile([H, T], F32, tag="ps", name=f"pe{g}")
                nc.tensor.matmul(pe, wencT_lat, latT_all[:, g, :], start=True, stop=False)
                nc.tensor.matmul(pe, wencT_gm, gmsT_all[:, g, :], start=False, stop=True)
                nc.vector.tensor_scalar_add(E_f[:, g, :], pe, benc)
                nc.vector.tensor_scalar_add(E_r[:, g, :], pe, benc)
                for c in range(4):
                    pm = pp.tile([H, T], F32, tag="ps", name=f"pm{g}_{c}")
                    nc.tensor.matmul(pm, wihT0[:, c * H:(c + 1) * H], E_r[:, g, :],
                                     start=True, stop=True)
                    nc.vector.tensor_scalar_add(M12[:, g, c, :], pm, gbias[:, c:c + 1])
                for c in range(4):
                    pm2 = pp.tile([H, T], F32, tag="ps", name=f"pm2{g}_{c}")
                    nc.tensor.matmul(pm2, wihT1gm[:, c * H:(c + 1) * H],
                                     gmsT_all[:, g, :], start=True, stop=True)
                    nc.vector.tensor_scalar_add(M12[:, g, 4 + c, :], pm2,
                                                gbias[:, 4 + c:5 + c])

            # ---- main block loop ----
            for b in range(NBLK):
                g = blkmap[b]
                nb = wp.tile([ND, B], F32R, tag="nb", bufs=2, name=f"nb{b}")
                nc.sync.dma_start(out=nb, in_=node_t_ext[:, b * B:(b + 1) * B])

                px = pp.tile([H, B], F32, tag="ps", name=f"px{b}")
                nc.tensor.matmul(px, wencT_node, nb, start=True, stop=True)
                enc_node = wp.tile([H, B], F32R, tag="encn", bufs=2, name=f"encn{b}")
                nc.vector.tensor_copy(enc_node, px)

                # persistent ping-pong state tiles for this block
                h1 = [wp.tile([H, B], F32R, tag=f"h1_{i}", bufs=2, name=f"h1_{i}_{b}") for i in range(2)]
                c1 = [wp.tile([H, B], F32, tag=f"c1_{i}", bufs=2, name=f"c1_{i}_{b}") for i in range(2)]
                h2 = [wp.tile([H, B], F32R, tag=f"h2_{i}", bufs=2, name=f"h2_{i}_{b}") for i in range(2)]
                c2 = [wp.tile([H, B], F32R, tag=f"c2_{i}", bufs=2, name=f"c2_{i}_{b}") for i in range(2)]
                for dst in (h1[0], c1[0], h2[0], c2[0]):
                    nc.vector.tensor_scalar_add(dst, px, E_f[:, g, 0:1])

                yacc = [wp.tile([H, T, GM], F32, tag=f"yacc{j}", bufs=2, name=f"yacc{j}_{b}")
                        for j in range(4)]
                stg = [wp.tile([H, 8], F32, tag=f"stg{i}", bufs=2, name=f"stg{i}_{b}")
                       for i in range(2)]

                with tc.For_i(0, T, UNROLL) as tv:
                    for k in range(UNROLL):
                        t = tv + k
                        pcur, pnxt = k % 2, 1 - (k % 2)
                        nc.vector.tensor_copy(stg[pcur], M12[:, g, :, DS(t, 1)].squeeze(-1))

                        # ---- layer 1 gates ----
                        acts1 = []
                        for c in range(4):
                            pg = pp.tile([H, B], F32, tag="ps", name=f"g1_{c}")
                            nc.tensor.matmul(pg, whhT0[:, c * H:(c + 1) * H], h1[pcur],
                                             start=True, stop=False)
                            nc.tensor.matmul(pg, wihT0[:, c * H:(c + 1) * H], enc_node,
                                             start=False, stop=True)
                            a = wp.tile([H, B], F32, tag=f"a1_{c}", bufs=2, name=f"a1_{c}")
                            nc.scalar.activation(out=a, in_=pg, func=GATE_FUNCS[c],
                                                 bias=stg[pcur][:, c:c + 1], scale=1.0)
                            acts1.append(a)
                        t1 = wp.tile([H, B], F32, tag="t1", bufs=2, name="t1")
                        t2 = wp.tile([H, B], F32, tag="t2", bufs=2, name="t2")
                        nc.vector.tensor_mul(t1, acts1[0], acts1[2])      # sig(i)*tanh(g)
                        nc.vector.tensor_mul(t2, acts1[1], c1[pcur])      # sig(f)*c
                        nc.vector.tensor_add(c1[pnxt], t1, t2)
                        tc1 = wp.tile([H, B], F32, tag="tc1", bufs=2, name="tc1")
                        nc.scalar.activation(out=tc1, in_=c1[pnxt], func=AF.Tanh)
                        nc.vector.tensor_mul(h1[pnxt], acts1[3], tc1)     # sig(o)*tanh(c)

                        # ---- layer 2 gates (input = [h1_new[:125]; gm] via M2 bias) ----
                        acts2 = []
                        for c in range(4):
                            pg2 = pp.tile([H, B], F32, tag="ps", name=f"g2_{c}")
                            nc.tensor.matmul(pg2, wihT1[0:H - GM, c * H:(c + 1) * H],
                                             h1[pnxt][0:H - GM, :], start=True, stop=False)
                            nc.tensor.matmul(pg2, whhT1[:, c * H:(c + 1) * H], h2[pcur],
                                             start=False, stop=True)
                            a2 = wp.tile([H, B], F32, tag=f"a2_{c}", bufs=2, name=f"a2_{c}")
                            nc.scalar.activation(out=a2, in_=pg2, func=GATE_FUNCS[c],
                                                 bias=stg[pcur][:, 4 + c:5 + c], scale=1.0)
                            acts2.append(a2)
                        t3 = wp.tile([H, B], F32, tag="t3", bufs=2, name="t3")
                        t4 = wp.tile([H, B], F32, tag="t4", bufs=2, name="t4")
                        nc.vector.tensor_mul(t3, acts2[0], acts2[2])
                        nc.vector.tensor_mul(t4, acts2[1], c2[pcur])
                        nc.vector.tensor_add(c2[pnxt], t3, t4)
                        tc2 = wp.tile([H, B], F32, tag="tc2", bufs=2, name="tc2")
                        nc.scalar.activation(out=tc2, in_=c2[pnxt], func=AF.Tanh)
                        nc.vector.tensor_mul(h2[pnxt], acts2[3], tc2)

                        # ---- decoder ----
                        pd = pp.tile([64, B], F32, tag="ps", name="pd")
                        nc.tensor.matmul(pd, wd1T[:, 0, :], h2[pnxt], start=True, stop=False)
                        nc.tensor.matmul(pd, wd1T[:, 1, :], c2[pnxt], start=False, stop=True)
                        relu = wp.tile([64, B], F32R, tag="relu", bufs=2, name="relu")
                        nc.scalar.activation(out=relu, in_=pd, func=AF.Relu, bias=bd1, scale=1.0)
                        py = pp.tile([H, 16], F32, tag="ps", name="py")
                        for j in range(4):
                            nc.tensor.matmul(py[:, j * 4:(j + 1) * 4],
                                             relu[:, j * H:(j + 1) * H], wd2T,
                                             start=True, stop=True)
                        for j in range(4):
                            nc.vector.tensor_add(
                                yacc[j][:, DS(t, 1), :].squeeze(1),
                                py[:, j * 4:j * 4 + GM], bd2rep)

                for j in range(4):
                    nc.sync.dma_start(out=y_ext[b * B + j * H:b * B + (j + 1) * H, :, :],
                                      in_=yacc[j])

    nc.finalize()
    return nc


_CACHE = {}
_LAST_IN_MAPS = None


def _get_nc(NBLK, NTAB, blkmap):
    key = (NBLK, NTAB, tuple(blkmap))
    if key not in _CACHE:
        _CACHE[key] = build_nc(NBLK, NTAB, blkmap)
    return _CACHE[key]


def kernel(node, ptr, graph_time_series_behavior, ground_motions,
           W_enc, b_enc, W_ih, W_hh, b_ih, b_hh, W_d1, b_d1, W_d2, b_d2):
    node = np.asarray(node, np.float32)
    ptr = np.asarray(ptr, np.int64)
    lat = np.asarray(graph_time_series_behavior, np.float32)
    gms = np.asarray(ground_motions, np.float32)
    W_enc = np.asarray(W_enc, np.float32); b_enc_a = np.asarray(b_enc, np.float32)
    W_ih = np.asarray(W_ih, np.float32); W_hh = np.asarray(W_hh, np.float32)
    b_ih = np.asarray(b_ih, np.float32); b_hh = np.asarray(b_hh, np.float32)
    W_d1 = np.asarray(W_d1, np.float32); b_d1_a = np.asarray(b_d1, np.float32)
    W_d2 = np.asarray(W_d2, np.float32); b_d2_a = np.asarray(b_d2, np.float32)

    N = node.shape[0]
    BS = lat.shape[0]
    gsizes = np.diff(ptr).astype(np.int64)
    assert gsizes.sum() == N

    # assign whole graphs to cores (contiguous groups)
    gper = (BS + NCORES - 1) // NCORES
    core_graphs = [list(range(c * gper, min((c + 1) * gper, BS))) for c in range(NCORES)]
    NTAB = max(len(cg) for cg in core_graphs)
    # per-core block -> tab map; blocks are per-graph padded to multiples of B
    core_blkmaps, core_nblk = [], []
    for cg in core_graphs:
        bm = []
        for slot, g in enumerate(cg):
            bm += [slot] * int((gsizes[g] + B - 1) // B)
        core_blkmaps.append(bm)
        core_nblk.append(len(bm))
    NBLK = max(core_nblk) if max(core_nblk) > 0 else 1
    for bm in core_blkmaps:
        bm += [0] * (NBLK - len(bm))
    if all(bm == core_blkmaps[0] for bm in core_blkmaps):
        blkmap = core_blkmaps[0]
        per_block_tabs = False
    else:
        # ragged fallback: one tab per block, latent replicated per block
        blkmap = list(range(NBLK))
        NTAB = NBLK
        per_block_tabs = True

    NPAD = NBLK * B
    # host-side permutation: per core, concatenated per-graph node ranges padded to B
    latT_w = np.ascontiguousarray(lat.transpose(0, 2, 1))   # [BS, H, T]
    gmsT_w = np.ascontiguousarray(gms.transpose(0, 2, 1))   # [BS, GM, T]

    in_maps = []
    core_index_maps = []
    for c, cg in enumerate(core_graphs):
        node_pad = np.zeros((NPAD, ND), np.float32)
        idx_map = np.full(NPAD, -1, np.int64)
        pos = 0
        for g in cg:
            s, e = int(ptr[g]), int(ptr[g + 1])
            n = e - s
            node_pad[pos:pos + n] = node[s:e]
            idx_map[pos:pos + n] = np.arange(s, e)
            pos += int((n + B - 1) // B) * B
        latT_c = np.zeros((NTAB, H, T), np.float32)
        gmsT_c = np.zeros((NTAB, GM, T), np.float32)
        if per_block_tabs:
            for bi, g_slot in enumerate(core_blkmaps[c][:core_nblk[c]]):
                pass
            # map each block to its graph directly
            bi = 0
            for g in cg:
                nblk_g = int((gsizes[g] + B - 1) // B)
                for _ in range(nblk_g):
                    latT_c[bi] = latT_w[g]
                    gmsT_c[bi] = gmsT_w[g]
                    bi += 1
        else:
            for slot, g in enumerate(cg):
                latT_c[slot] = latT_w[g]
                gmsT_c[slot] = gmsT_w[g]
        in_maps.append(dict(
            node_t=np.ascontiguousarray(node_pad.T),
            latT=latT_c, gmsT=gmsT_c,
            wencT_node=np.ascontiguousarray(W_enc[:, :ND].T),
            wencT_lat=np.ascontiguousarray(W_enc[:, ND:ND + H].T),
            wencT_gm=np.ascontiguousarray(W_enc[:, ND + H:].T),
            wihT0=np.ascontiguousarray(W_ih[0].T),
            whhT0=np.ascontiguousarray(W_hh[0].T),
            wihT1=np.ascontiguousarray(W_ih[1].T),
            wihT1gm=np.ascontiguousarray(W_ih[1][:, H - GM:].T),
            whhT1=np.ascontiguousarray(W_hh[1].T),
            wd1T=np.ascontiguousarray(np.stack([W_d1[:, :H].T, W_d1[:, H:].T], axis=1)),
            wd2T=np.ascontiguousarray(np.concatenate([W_d2.T, np.zeros((64, 1), np.float32)], 1)),
            gbias=np.ascontiguousarray(
                np.concatenate([(b_ih[0] + b_hh[0]).reshape(4, H),
                                (b_ih[1] + b_hh[1]).reshape(4, H)], 0).T),
            benc=b_enc_a.reshape(H, 1),
            bd1=b_d1_a.reshape(64, 1),
            bd2rep=np.ascontiguousarray(np.broadcast_to(b_d2_a, (H, GM))),
        ))
        core_index_maps.append(idx_map)

    global _LAST_IN_MAPS
    _LAST_IN_MAPS = in_maps
    nc = _get_nc(NBLK, NTAB, blkmap)
    res = run_bass_kernel_spmd(nc, in_maps, list(range(NCORES)))

    out = np.empty((N, T, GM), np.float32)
    for c in range(NCORES):
        y = res.results[c]["y"]
        m = core_index_maps[c]
        valid = m >= 0
        out[m[valid]] = y[valid]
    return out
